# revision 29
# baseline (speedup 1.0000x reference)
"""LSTM cell (B=4096, D=U=2048) on 8 trn2 NeuronCores.

Tensor-parallel over units: core i computes units [i*256,(i+1)*256) of every
gate. Per core:
    z^T[1024 units, 4096 batch] = Wx_shard^T @ x^T + Wh_shard^T @ h^T
accumulated in PSUM, gate activations fused with the bias add on ScalarE
(units on partitions -> bias is per-partition), elementwise LSTM combine on
VectorE, outputs stored transposed and re-transposed on the host.

v20: per-gate precision allocation. Quantization noise injected into a
gate's pre-activation is amplified ~6.5x more by g (tanh, unit slope) than
by i (sigmoid, slope <= 1/4, times tanh'(c_new) attenuation), so the fp8
budget goes where it is cheap: f and i run their entire 4096-deep
contraction in fp8 e4m3 DoubleRow (2 k-tiles per PE pass), o runs bf16 with
its last 6 h k-tiles fp8, g stays all-bf16. The summed per-gate noise
variances land at rel err 1.934e-2 (budget 2e-2, exact numpy sim of the
quantization matches hardware to 4 digits), while the matmul instruction
count drops from 29 to 23.25 per (gate,ut,btile) on average.

Schedule: the DMA engine delivers only ~115GB/s for its first ~13us (it
has its own ramp), so batch tile 0 opens with the f/i fp8 x-phase, whose
head (x8 + wxq) is written in 512KB slice-granular pieces — Tile tracks
AP-overlap deps, so the first DoubleRow pair starts after the first two
slices instead of the whole 2.5MB head — and its ~7us of work gives the
bf16 x+weight stream a head start; the bf16 k-outer then tracks chunk
arrivals across all 8 PSUM banks with HAM staying at full rate end-to-end. Tiles 2..7 run in weight-stationary pairs with two 4-bank PSUM
waves (g,i then f,o) so wave evacuation overlaps the other wave's matmuls;
the final o-gate is split 384/128 so only a 128-wide act+mul+store trails
the last matmul. DMA dispatch costs ~0.7us per descriptor-set on an engine
queue, so loads are batched (4-ktile chunks) and split across the two HWDGE
queues by consumption time: sync carries the fp8 x head, then the n0 act
chunks with the x weights interleaved right behind their chunks (arrival
order == consumption order on one FIFO, robust to cross-queue bandwidth
races), then the late-consumed fp8 h tail (whq/who8/h8); scalar carries the remaining
weight stream (it is idle until the first elementwise); per-batch-tile act
loads all ride sync, since scalar-queue dispatches would serialize behind
stage activations.
"""

import sys

sys.path.insert(0, "/opt/trn_rl_repo")

import ml_dtypes
import numpy as np

import concourse.bass as bass
import concourse.mybir as mybir
import concourse.tile as tile
from concourse.bass_utils import run_bass_kernel_spmd

B, D, U = 4096, 2048, 2048
N_CORES = 8
US = U // N_CORES          # units per core per gate (256)
UT = US // 128             # unit tiles of 128 per gate (2)
NB = 512                   # batch tile (free dim)
NT = B // NB               # batch tiles (8)
KX = D // 128              # k tiles for x gemm (16)
KH = U // 128              # k tiles for h gemm (16)
XP = KX // 2               # fp8 DoubleRow pairs over x (8)
HP = KH // 2               # fp8 DoubleRow pairs over h (8)
OBH = 10                   # o-gate bf16 h k-tiles; tiles 10..15 run fp8
OBW = 12                   # whb batch coverage ([o|g] cols); g-only beyond
BF16 = mybir.dt.bfloat16
FP8 = mybir.dt.float8e4
F32 = mybir.dt.float32
AF = mybir.ActivationFunctionType
DR = mybir.MatmulPerfMode.DoubleRow
S8 = 8.0                   # symmetric fp8 scale: act/8 and W*8 (cancels)

# gate index in the elementwise/bias layout: f=0, i=1, o=2, g=3
GATE_ORDER = (3, 1, 0, 2)  # consumption order g, i, f, o


def _split_excess_waits(nc, maxw=1):
    """This walrus build rejects instructions carrying more than one sem-wait
    ("Too many sync wait commands"), but Tile freely attaches several. Hoist
    the extra waits onto same-engine nops inserted right before the
    instruction — engine streams are in-order, so blocking semantics are
    identical."""
    cnt = 0
    for fn in nc.m.functions:
        for bb in fn.blocks:
            new_insts = []
            for inst in bb.instructions:
                si = inst.sync_info
                waits = list(si.on_wait) if si is not None else []
                if len(waits) > maxw:
                    for i in range(0, len(waits) - maxw, maxw):
                        nop = mybir.InstNoOp(name=f"syncsplit-{cnt}")
                        cnt += 1
                        nop.engine = inst.engine
                        nop.sync_info = mybir.SyncInfo(
                            on_wait=waits[i : i + maxw], on_update=[]
                        )
                        new_insts.append(nop)
                    si.on_wait = waits[len(waits) - maxw :]
                new_insts.append(inst)
            if len(new_insts) != len(bb.instructions):
                bb.instructions = new_insts
    return cnt


def build_nc() -> bass.Bass:
    nc = bass.Bass()

    xT = nc.dram_tensor("xT", [D, B], BF16, kind="ExternalInput")
    hT = nc.dram_tensor("hT", [U, B], BF16, kind="ExternalInput")
    xT8 = nc.dram_tensor("xT8", [D, B], FP8, kind="ExternalInput")
    hT8 = nc.dram_tensor("hT8", [U, B], FP8, kind="ExternalInput")
    # bf16 weights, columns [o | g] (256 each); o rows used only for kt<OBH
    wxb = nc.dram_tensor("wxb", [D, 2 * US], BF16, kind="ExternalInput")
    whb = nc.dram_tensor("whb", [U, 2 * US], BF16, kind="ExternalInput")
    # fp8 weights, columns [f | i]; host pre-scales W*8, act/8 (exact cancel)
    wxq = nc.dram_tensor("wxq", [D, 2 * US], FP8, kind="ExternalInput")
    whq = nc.dram_tensor("whq", [U, 2 * US], FP8, kind="ExternalInput")
    who8 = nc.dram_tensor("who8", [(KH - OBH) * 128, US], FP8, kind="ExternalInput")
    # bias, host-prepped to [128, 8]: column j = units [j*128,(j+1)*128) of
    # the concatenated [f,i,o,g] 1024-unit block (gate j//2, unit-tile j%2)
    bias = nc.dram_tensor("bias", [128, 4 * UT], F32, kind="ExternalInput")
    # c input and both outputs travel as bf16: halves DMA traffic and the
    # final output drain; costs ~1e-3 rel err (budget is 2e-2)
    cT = nc.dram_tensor("cT", [US, B], BF16, kind="ExternalInput")
    h_newT = nc.dram_tensor("h_newT", [US, B], BF16, kind="ExternalOutput")
    c_newT = nc.dram_tensor("c_newT", [US, B], BF16, kind="ExternalOutput")

    xT_r = xT.rearrange("(kt p) b -> p kt b", p=128)    # [128, KX, B]
    hT_r = hT.rearrange("(kt p) b -> p kt b", p=128)
    xT8_r = xT8.rearrange("(kt p) b -> p kt b", p=128)
    hT8_r = hT8.rearrange("(kt p) b -> p kt b", p=128)
    wxb_r = wxb.rearrange("(kt p) u -> p kt u", p=128)  # [128, KX, 512]
    whb_r = whb.rearrange("(kt p) u -> p kt u", p=128)
    wxq_r = wxq.rearrange("(kt p) u -> p kt u", p=128)
    whq_r = whq.rearrange("(kt p) u -> p kt u", p=128)
    who8_r = who8.rearrange("(kt p) u -> p kt u", p=128)  # [128, 4, US]

    XCH = [(0, 1), (1, 2), (2, 4), (4, 8), (8, 12), (12, 16)]
    HCH = [(0, 4), (4, 8), (8, 12), (12, 16)]

    with tile.TileContext(nc) as tc:
        with (
            tc.tile_pool(name="wpool", bufs=1) as wpool,
            tc.tile_pool(name="singles", bufs=1) as singles,
            tc.tile_pool(name="acts", bufs=2) as apool,
            tc.tile_pool(name="ew", bufs=2) as epool,
            tc.tile_pool(name="psum", bufs=8, space="PSUM") as ppool,
        ):
            # --- weight column helpers. Weights stream in 4-ktile batches
            # on the Activation-engine HWDGE queue so their dispatch cost
            # never serializes with the activation loads on the sync queue.
            wxb0_t = [None] * 4           # kt 0..3 as single-kt tiles
            wxbB = [None] * (KX // 4)
            whbB = [None] * (OBW // 4)
            whgB = [None]                 # g-only cols for kt >= OBH
            wxqB = [None] * 2
            whqB = [None] * 2

            def bcol(gi, ut):  # o,g in wxb/whb
                return (0 if gi == 2 else US) + ut * 128

            def qcol(gi, ut):  # f,i in wxq/whq
                return (0 if gi == 0 else US) + ut * 128

            def wxb_ap(gi, ut, kt):
                c0 = bcol(gi, ut)
                if kt < 4:
                    return wxb0_t[kt][:, c0 : c0 + 128]
                return wxbB[kt // 4][:, kt % 4, c0 : c0 + 128]

            def whb_ap(ut, kt):  # g-gate h weight
                if kt < OBW:
                    c0 = bcol(3, ut)
                    return whbB[kt // 4][:, kt % 4, c0 : c0 + 128]
                c0 = ut * 128
                return whgB[0][:, kt - OBW, c0 : c0 + 128]

            def who_ap(ut, kt):  # o-gate bf16 h weight, kt < OBH
                c0 = bcol(2, ut)
                return whbB[kt // 4][:, kt % 4, c0 : c0 + 128]

            def wxq_ap(gi, ut, p):
                c0 = qcol(gi, ut)
                return wxqB[p // 4][:, 2 * (p % 4) : 2 * (p % 4) + 2, c0 : c0 + 128]

            def whq_ap(gi, ut, p):
                c0 = qcol(gi, ut)
                return whqB[p // 4][:, 2 * (p % 4) : 2 * (p % 4) + 2, c0 : c0 + 128]

            # --- startup DMA stream (single HWDGE ring, FIFO) in PE
            # consumption order: bf16 x chunks + their weights, then the fp8
            # x copy + weights, then h likewise.
            # the DMA engine runs at ~1/3 rate for its first ~13us, so the
            # fp8 head is written in slice-granular pieces: the first DR
            # pair needs only the first 512KB, not the whole 2.5MB head
            x8_0 = apool.tile([128, KX, NB], FP8, tag="x8_sb", name="x8_0")
            wxqB[0] = wpool.tile([128, 8, 2 * US], FP8, tag="wxqB0", name="wxqB0")
            wxqB[1] = wpool.tile([128, 8, 2 * US], FP8, tag="wxqB1", name="wxqB1")
            nc.sync.dma_start(out=x8_0[:, 0:2, :], in_=xT8_r[:, 0:2, bass.ts(0, NB)])
            nc.sync.dma_start(out=wxqB[0][:, 0:2, :], in_=wxq_r[:, 0:2, :])
            nc.sync.dma_start(out=x8_0[:, 2:4, :], in_=xT8_r[:, 2:4, bass.ts(0, NB)])
            nc.sync.dma_start(out=wxqB[0][:, 2:4, :], in_=wxq_r[:, 2:4, :])
            nc.sync.dma_start(out=x8_0[:, 4:8, :], in_=xT8_r[:, 4:8, bass.ts(0, NB)])
            nc.sync.dma_start(out=wxqB[0][:, 4:8, :], in_=wxq_r[:, 4:8, :])
            nc.sync.dma_start(out=x8_0[:, 8:12, :], in_=xT8_r[:, 8:12, bass.ts(0, NB)])
            nc.sync.dma_start(out=wxqB[1][:, 0:4, :], in_=wxq_r[:, 8:12, :])
            nc.sync.dma_start(out=x8_0[:, 12:16, :], in_=xT8_r[:, 12:16, bass.ts(0, NB)])
            nc.sync.dma_start(out=wxqB[1][:, 4:8, :], in_=wxq_r[:, 12:16, :])
            x0c = {}
            for (k0, k1) in XCH:
                xc = apool.tile(
                    [128, k1 - k0, NB], BF16, tag=f"xbc{k0}", bufs=1, name=f"x0c{k0}"
                )
                nc.sync.dma_start(out=xc[:], in_=xT_r[:, k0:k1, bass.ts(0, NB)])
                for kt in range(k0, k1):
                    x0c[kt] = xc[:, kt - k0, :]
                # x weights ride the same FIFO right behind their chunk, so
                # arrival order equals consumption order with no cross-queue
                # bandwidth race
                if k1 <= 2:
                    for kt in range(k0, k1):
                        wt = wpool.tile([128, 2 * US], BF16, tag=f"wxb0{kt}")
                        nc.sync.dma_start(out=wt[:], in_=wxb_r[:, kt, :])
                        wxb0_t[kt] = wt
                elif k1 <= 4:
                    wt = wpool.tile([128, 2, 2 * US], BF16, tag="wxb023")
                    nc.sync.dma_start(out=wt[:], in_=wxb_r[:, 2:4, :])
                    wxb0_t[2] = wt[:, 0, :]
                    wxb0_t[3] = wt[:, 1, :]
                else:
                    j = k0 // 4
                    wt = wpool.tile(
                        [128, 4, 2 * US], BF16, tag=f"wxbB{j}", name=f"wxbB{j}"
                    )
                    nc.sync.dma_start(
                        out=wt[:, 0:2, :], in_=wxb_r[:, k0 : k0 + 2, :]
                    )
                    nc.sync.dma_start(
                        out=wt[:, 2:4, :], in_=wxb_r[:, k0 + 2 : k0 + 4, :]
                    )
                    wxbB[j] = wt
            h0c = {}
            for (k0, k1) in HCH:
                hc = apool.tile(
                    [128, k1 - k0, NB], BF16, tag=f"hbc{k0}", bufs=1, name=f"h0c{k0}"
                )
                nc.sync.dma_start(out=hc[:], in_=hT_r[:, k0:k1, bass.ts(0, NB)])
                for kt in range(k0, k1):
                    h0c[kt] = hc[:, kt - k0, :]
            # remaining (later-consumed) weights stream on the scalar queue
            b_sb = singles.tile([128, 4 * UT], F32)
            nc.scalar.dma_start(out=b_sb[:], in_=bias[:])
            for j in range(OBW // 4):
                wt = wpool.tile([128, 4, 2 * US], BF16, tag=f"whbB{j}")
                nc.scalar.dma_start(out=wt[:], in_=whb_r[:, 4 * j : 4 * j + 4, :])
                whbB[j] = wt
            wt = wpool.tile([128, KH - OBW, US], BF16, tag="whgB")
            nc.scalar.dma_start(out=wt[:], in_=whb_r[:, OBW:KH, US : 2 * US])
            whgB[0] = wt
            # the fp8 h-phase tail (i/f h weights, o8 weights, h8 acts) is
            # consumed last in n0; it rides the sync queue, which drains its
            # startup work ~20us before the PE needs these
            for j in range(2):
                wt = wpool.tile([128, 8, 2 * US], FP8, tag=f"whqB{j}")
                nc.sync.dma_start(out=wt[:], in_=whq_r[:, 8 * j : 8 * j + 8, :])
                whqB[j] = wt
            who8_t = wpool.tile([128, KH - OBH, US], FP8, tag="who8")
            nc.sync.dma_start(out=who8_t[:], in_=who8_r[:])
            h8_0 = apool.tile([128, KH, NB], FP8, tag="h8_sb", name="h8_0")
            nc.sync.dma_start(out=h8_0[:], in_=hT8_r[:, :, bass.ts(0, NB)])

            # --- matmul emitters. Moving operands come via accessors:
            # xb(kt)/hb(kt) -> [128, NB] bf16, x8(p)/h8(p) -> [128, 2, NB].
            def mm_g(ps, ut, xb, hb, cols=slice(0, NB)):
                for kt in range(KX):
                    nc.tensor.matmul(
                        ps, wxb_ap(3, ut, kt), xb(kt)[:, cols],
                        start=(kt == 0), stop=False,
                    )
                for kt in range(KH):
                    nc.tensor.matmul(
                        ps, whb_ap(ut, kt), hb(kt)[:, cols],
                        start=False, stop=(kt == KH - 1),
                    )

            def mm_o_x(ps, ut, xb, cols=slice(0, NB)):
                for kt in range(KX):
                    nc.tensor.matmul(
                        ps, wxb_ap(2, ut, kt), xb(kt)[:, cols],
                        start=(kt == 0), stop=False,
                    )

            def mm_o_hb(ps, ut, hb, cols=slice(0, NB)):
                for kt in range(OBH):
                    nc.tensor.matmul(
                        ps, who_ap(ut, kt), hb(kt)[:, cols],
                        start=False, stop=False,
                    )

            def mm_o_h(ps, ut, hb, h8, cols=slice(0, NB)):
                mm_o_hb(ps, ut, hb, cols)
                mm_o8(ps, ut, h8, cols)

            def mm_o8(ps, ut, h8, cols=slice(0, NB)):
                co = ut * 128
                for j in range((KH - OBH) // 2):
                    nc.tensor.matmul(
                        ps,
                        who8_t[:, 2 * j : 2 * j + 2, co : co + 128],
                        h8(HP - (KH - OBH) // 2 + j)[:, :, cols],
                        start=False, stop=(j == (KH - OBH) // 2 - 1),
                        perf_mode=DR,
                    )

            def mm_q_x(ps, gi, ut, x8, cols=slice(0, NB)):
                for p in range(XP):
                    nc.tensor.matmul(
                        ps, wxq_ap(gi, ut, p), x8(p)[:, :, cols],
                        start=(p == 0), stop=False, perf_mode=DR,
                    )

            def mm_q_h(ps, gi, ut, h8, cols=slice(0, NB)):
                for p in range(HP):
                    nc.tensor.matmul(
                        ps, whq_ap(gi, ut, p), h8(p)[:, :, cols],
                        start=False, stop=(p == HP - 1), perf_mode=DR,
                    )

            def act_gate(ps, gi, ut, name, w=NB):
                # o-gate tiles are bf16 (they feed the bf16 h_new output);
                # f/i/g stay fp32 for the c_new accumulate path
                dt = BF16 if gi == 2 else F32
                g_sb = epool.tile([128, w], dt, tag=f"gate{gi}_{w}", name=name)
                nc.scalar.activation(
                    g_sb[:],
                    ps[:],
                    AF.Tanh if gi == 3 else AF.Sigmoid,
                    bias=b_sb[:, gi * UT + ut : gi * UT + ut + 1],
                )
                return g_sb

            def elementwise(pss, n, ut):
                # pss indexed by gi; groups complete in GATE_ORDER, so
                # evaluate the LSTM chain in that order
                nsl = bass.ts(n, NB)
                usl = slice(ut * 128, (ut + 1) * 128)
                c_sb = epool.tile([128, NB], BF16, tag="c_sb", name="c_sb")
                nc.sync.dma_start(out=c_sb[:], in_=cT[usl, nsl])
                g_t = act_gate(pss[3], 3, ut, "g_t")
                i_t = act_gate(pss[1], 1, ut, "i_t")
                nc.vector.tensor_mul(i_t[:], i_t[:], g_t[:])      # i*g
                f_t = act_gate(pss[0], 0, ut, "f_t")
                nc.vector.tensor_mul(f_t[:], f_t[:], c_sb[:])     # f*c
                cn = epool.tile([128, NB], BF16, tag="cn", name="cn")
                nc.vector.tensor_add(cn[:], f_t[:], i_t[:])       # c_new
                nc.sync.dma_start(out=c_newT[usl, nsl], in_=cn[:])
                tn = epool.tile([128, NB], BF16, tag="tnb", name="tn")
                nc.scalar.activation(tn[:], cn[:], AF.Tanh)       # tanh(c_new)
                o_t = act_gate(pss[2], 2, ut, "o_t")
                nc.vector.tensor_mul(o_t[:], o_t[:], tn[:])       # h_new
                nc.sync.dma_start(out=h_newT[usl, nsl], in_=o_t[:])

            def stage1(psA, ti, ut, tname):
                # after wave A (g,i) stops: compute ig = sigmoid(i)*tanh(g),
                # freeing wave A's PSUM banks while wave B still matmuls
                g_t = epool.tile([128, NB], F32, tag="gate3", name=f"g_{tname}")
                nc.scalar.activation(
                    g_t[:], psA[3][ti][:], AF.Tanh,
                    bias=b_sb[:, 3 * UT + ut : 3 * UT + ut + 1],
                )
                ig = epool.tile([128, NB], F32, tag="ig", name=f"ig_{tname}")
                nc.scalar.activation(
                    ig[:], psA[1][ti][:], AF.Sigmoid,
                    bias=b_sb[:, 1 * UT + ut : 1 * UT + ut + 1],
                )
                nc.vector.tensor_mul(ig[:], ig[:], g_t[:])
                return ig

            def stage2(psB, ti, ig, n, ut):
                # after wave B (f,o) stops: finish the LSTM combine
                nsl = bass.ts(n, NB)
                usl = slice(ut * 128, (ut + 1) * 128)
                c_sb = epool.tile([128, NB], BF16, tag="c_sb", name="c_sb")
                nc.sync.dma_start(out=c_sb[:], in_=cT[usl, nsl])
                f_t = act_gate(psB[0][ti], 0, ut, "f_t")
                nc.vector.tensor_mul(f_t[:], f_t[:], c_sb[:])     # f*c
                cn = epool.tile([128, NB], BF16, tag="cn", name="cn")
                nc.vector.tensor_add(cn[:], f_t[:], ig[:])        # c_new
                nc.sync.dma_start(out=c_newT[usl, nsl], in_=cn[:])
                tn = epool.tile([128, NB], BF16, tag="tnb", name="tn")
                nc.scalar.activation(tn[:], cn[:], AF.Tanh)       # tanh(c_new)
                o_t = act_gate(psB[2][ti], 2, ut, "o_t")
                nc.vector.tensor_mul(o_t[:], o_t[:], tn[:])       # h_new
                nc.sync.dma_start(out=h_newT[usl, nsl], in_=o_t[:])

            # --- n = 0: k-outer over the bf16 gates across all 8 PSUM banks
            # so the PE tracks the chunked arrival stream; the fp8 gates'
            # DoubleRow matmuls run after (their whole-tile deps have landed
            # by then).
            ps_all = [
                [
                    ppool.tile([128, NB], F32, tag="ps", name=f"ps{ut}{gi}")
                    for gi in range(4)
                ]
                for ut in range(UT)
            ]
            xb0 = lambda kt: x0c[kt]
            hb0 = lambda kt: h0c[kt]
            x80 = lambda p: x8_0[:, 2 * p : 2 * p + 2, :]
            h80 = lambda p: h8_0[:, 2 * p : 2 * p + 2, :]
            # fp8 x-phase first: it needs only 1.5MB to start, and its
            # ~7us of work gives the bf16 x+weight FIFO a head start so the
            # k-outer phase below never catches the stream
            for p in range(XP):
                for gi in (1, 0):
                    for ut in range(UT):
                        nc.tensor.matmul(
                            ps_all[ut][gi][:],
                            wxq_ap(gi, ut, p), x80(p),
                            start=(p == 0), stop=False, perf_mode=DR,
                        )
            for kt in range(KX):
                for ut in range(UT):
                    nc.tensor.matmul(
                        ps_all[ut][3][:], wxb_ap(3, ut, kt), x0c[kt],
                        start=(kt == 0), stop=False,
                    )
                for ut in range(UT):
                    nc.tensor.matmul(
                        ps_all[ut][2][:], wxb_ap(2, ut, kt), x0c[kt],
                        start=(kt == 0), stop=False,
                    )
            for kt in range(OBH):
                for ut in range(UT):
                    nc.tensor.matmul(
                        ps_all[ut][3][:], whb_ap(ut, kt), h0c[kt],
                        start=False, stop=False,
                    )
                for ut in range(UT):
                    nc.tensor.matmul(
                        ps_all[ut][2][:], who_ap(ut, kt), h0c[kt],
                        start=False, stop=False,
                    )
            for kt in range(OBH, KH):
                for ut in range(UT):
                    nc.tensor.matmul(
                        ps_all[ut][3][:], whb_ap(ut, kt), h0c[kt],
                        start=False, stop=(kt == KH - 1),
                    )
            for gi in (1, 0):
                for ut in range(UT):
                    mm_q_h(ps_all[ut][gi][:], gi, ut, h80)
            for ut in range(UT):
                mm_o8(ps_all[ut][2][:], ut, h80)

            # --- n = 1 loads issued before n0's elementwise DMAs so they
            # don't queue behind the output drains on the ring. bf16 x/h
            # reuse the n0 chunk buffers (their readers are all emitted);
            # fp8 copies take the second buffer of the pair tags.
            x1c = {}
            for (k0, k1) in XCH:
                xc = apool.tile(
                    [128, k1 - k0, NB], BF16, tag=f"xbc{k0}", bufs=1, name=f"x1c{k0}"
                )
                nc.sync.dma_start(out=xc[:], in_=xT_r[:, k0:k1, bass.ts(1, NB)])
                for kt in range(k0, k1):
                    x1c[kt] = xc[:, kt - k0, :]
            h1c = {}
            for (k0, k1) in HCH:
                hc = apool.tile(
                    [128, k1 - k0, NB], BF16, tag=f"hbc{k0}", bufs=1, name=f"h1c{k0}"
                )
                nc.sync.dma_start(out=hc[:], in_=hT_r[:, k0:k1, bass.ts(1, NB)])
                for kt in range(k0, k1):
                    h1c[kt] = hc[:, kt - k0, :]
            x8_1 = apool.tile([128, KX, NB], FP8, tag="x8_sb", name="x8_1")
            nc.sync.dma_start(out=x8_1[:], in_=xT8_r[:, :, bass.ts(1, NB)])
            h8_1 = apool.tile([128, KH, NB], FP8, tag="h8_sb", name="h8_1")
            nc.sync.dma_start(out=h8_1[:], in_=hT8_r[:, :, bass.ts(1, NB)])

            for ut in range(UT):
                elementwise(ps_all[ut], 0, ut)

            xb1 = lambda kt: x1c[kt]
            hb1 = lambda kt: h1c[kt]
            x81 = lambda p: x8_1[:, 2 * p : 2 * p + 2, :]
            h81 = lambda p: h8_1[:, 2 * p : 2 * p + 2, :]
            for ut in range(UT):
                pss = [
                    ppool.tile([128, NB], F32, tag="ps", name=f"ps{gi}")
                    for gi in range(4)
                ]
                mm_g(pss[3][:], ut, xb1, hb1)
                mm_q_x(pss[1][:], 1, ut, x81)
                mm_q_h(pss[1][:], 1, ut, h81)
                mm_q_x(pss[0][:], 0, ut, x81)
                mm_q_h(pss[0][:], 0, ut, h81)
                mm_o_x(pss[2][:], ut, xb1)
                mm_o_h(pss[2][:], ut, hb1, h81)
                elementwise(pss, 1, ut)

            # --- n = 2..7 in weight-stationary pairs: per stationary weight
            # slice, both batch tiles' matmuls run back-to-back (walrus skips
            # the second LDWEIGHTS). Two 4-bank PSUM waves per ut: A=(g,i),
            # B=(f,o); both waves' x phases run first so the pair's x tiles
            # die early enough for the next pair's prefetch.
            WAVE_A = (3, 1)
            WAVE_B = (0, 2)
            for (na, nb) in ((2, 3), (4, 5), (6, 7)):
                last_pair = nb == NT - 1
                xa_t = apool.tile([128, KX, NB], BF16, tag="x_sb", name=f"x{na}")
                nc.sync.dma_start(out=xa_t[:], in_=xT_r[:, :, bass.ts(na, NB)])
                x8a_t = apool.tile([128, KX, NB], FP8, tag="x8_sb", name=f"x8{na}")
                nc.sync.dma_start(out=x8a_t[:], in_=xT8_r[:, :, bass.ts(na, NB)])
                xb_t = apool.tile([128, KX, NB], BF16, tag="x_sb", name=f"x{nb}")
                nc.sync.dma_start(out=xb_t[:], in_=xT_r[:, :, bass.ts(nb, NB)])
                x8b_t = apool.tile([128, KX, NB], FP8, tag="x8_sb", name=f"x8{nb}")
                nc.sync.dma_start(out=x8b_t[:], in_=xT8_r[:, :, bass.ts(nb, NB)])
                ha_t = apool.tile([128, KH, NB], BF16, tag="h_sb", name=f"h{na}")
                nc.sync.dma_start(out=ha_t[:], in_=hT_r[:, :, bass.ts(na, NB)])
                h8a_t = apool.tile([128, KH, NB], FP8, tag="h8_sb", name=f"h8{na}")
                nc.sync.dma_start(out=h8a_t[:], in_=hT8_r[:, :, bass.ts(na, NB)])
                hb_t = apool.tile([128, KH, NB], BF16, tag="h_sb", name=f"h{nb}")
                nc.sync.dma_start(out=hb_t[:], in_=hT_r[:, :, bass.ts(nb, NB)])
                h8b_t = apool.tile([128, KH, NB], FP8, tag="h8_sb", name=f"h8{nb}")
                nc.sync.dma_start(out=h8b_t[:], in_=hT8_r[:, :, bass.ts(nb, NB)])
                xab = [lambda kt, t=xa_t: t[:, kt, :], lambda kt, t=xb_t: t[:, kt, :]]
                hab = [lambda kt, t=ha_t: t[:, kt, :], lambda kt, t=hb_t: t[:, kt, :]]
                x8ab = [
                    lambda p, t=x8a_t: t[:, 2 * p : 2 * p + 2, :],
                    lambda p, t=x8b_t: t[:, 2 * p : 2 * p + 2, :],
                ]
                h8ab = [
                    lambda p, t=h8a_t: t[:, 2 * p : 2 * p + 2, :],
                    lambda p, t=h8b_t: t[:, 2 * p : 2 * p + 2, :],
                ]

                for ut in range(UT):
                    if last_pair and ut == UT - 1:
                        # kernel finale: per-tile gate-outer; tile b's o-gate
                        # is split 384/128 so only a 128-wide act+mul+store
                        # trails the last matmul
                        usl = slice(ut * 128, (ut + 1) * 128)
                        pss = [
                            ppool.tile([128, NB], F32, tag="ps", name=f"ps{gi}")
                            for gi in range(4)
                        ]
                        mm_g(pss[3][:], ut, xab[0], hab[0])
                        mm_q_x(pss[1][:], 1, ut, x8ab[0])
                        mm_q_h(pss[1][:], 1, ut, h8ab[0])
                        mm_q_x(pss[0][:], 0, ut, x8ab[0])
                        mm_q_h(pss[0][:], 0, ut, h8ab[0])
                        mm_o_x(pss[2][:], ut, xab[0])
                        mm_o_h(pss[2][:], ut, hab[0], h8ab[0])
                        elementwise(pss, na, ut)

                        nslb = bass.ts(nb, NB)
                        psb = [
                            ppool.tile([128, NB], F32, tag="ps", name=f"psb{gi}")
                            for gi in range(4)
                        ]
                        mm_g(psb[3][:], ut, xab[1], hab[1])
                        mm_q_x(psb[1][:], 1, ut, x8ab[1])
                        mm_q_h(psb[1][:], 1, ut, h8ab[1])
                        mm_q_x(psb[0][:], 0, ut, x8ab[1])
                        mm_q_h(psb[0][:], 0, ut, h8ab[1])
                        # combine chain for c_new runs during the o loops
                        c_sb = epool.tile([128, NB], BF16, tag="c_sb", name="c_sb")
                        nc.sync.dma_start(out=c_sb[:], in_=cT[usl, nslb])
                        g_t = act_gate(psb[3], 3, ut, "g_t")
                        i_t = act_gate(psb[1], 1, ut, "i_t")
                        nc.vector.tensor_mul(i_t[:], i_t[:], g_t[:])
                        f_t = act_gate(psb[0], 0, ut, "f_t")
                        nc.vector.tensor_mul(f_t[:], f_t[:], c_sb[:])
                        cn = epool.tile([128, NB], BF16, tag="cn", name="cn")
                        nc.vector.tensor_add(cn[:], f_t[:], i_t[:])
                        nc.sync.dma_start(out=c_newT[usl, nslb], in_=cn[:])
                        tn = epool.tile([128, NB], BF16, tag="tnb", name="tn")
                        nc.scalar.activation(tn[:], cn[:], AF.Tanh)
                        # o gate, wide part: evacuates while the narrow part
                        # is still matmuling
                        c1 = slice(0, 384)
                        mm_o_x(psb[2][:, c1], ut, xab[1], cols=c1)
                        mm_o_h(psb[2][:, c1], ut, hab[1], h8ab[1], cols=c1)
                        o1 = act_gate(psb[2][:, c1], 2, ut, "o1", w=384)
                        nc.vector.tensor_mul(o1[:], o1[:], tn[:, c1])
                        nc.sync.dma_start(
                            out=h_newT[usl, nb * NB : nb * NB + 384], in_=o1[:]
                        )
                        ps_o2 = ppool.tile([128, NB], F32, tag="ps", name="ps_o2")
                        c2 = slice(384, 512)
                        mm_o_x(ps_o2[:, 0:128], ut, xab[1], cols=c2)
                        mm_o_h(ps_o2[:, 0:128], ut, hab[1], h8ab[1], cols=c2)
                        o2 = act_gate(ps_o2[:, 0:128], 2, ut, "o2", w=128)
                        nc.vector.tensor_mul(o2[:], o2[:], tn[:, c2])
                        nc.sync.dma_start(
                            out=h_newT[usl, nb * NB + 384 : (nb + 1) * NB],
                            in_=o2[:],
                        )
                        continue
                    psA = {
                        gi: [
                            ppool.tile([128, NB], F32, tag="ps", name=f"ps{gi}{t}")
                            for t in "ab"
                        ]
                        for gi in WAVE_A
                    }
                    # wave A x phase: g (bf16) + i (fp8 DR), weight-stationary
                    for p in range(XP):
                        for kt in (2 * p, 2 * p + 1):
                            w = wxb_ap(3, ut, kt)
                            for ti in range(2):
                                nc.tensor.matmul(
                                    psA[3][ti][:], w, xab[ti](kt),
                                    start=(kt == 0), stop=False,
                                )
                        wq = wxq_ap(1, ut, p)
                        for ti in range(2):
                            nc.tensor.matmul(
                                psA[1][ti][:], wq, x8ab[ti](p),
                                start=(p == 0), stop=False, perf_mode=DR,
                            )
                    psB = {
                        gi: [
                            ppool.tile([128, NB], F32, tag="ps", name=f"ps{gi}{t}")
                            for t in "ab"
                        ]
                        for gi in WAVE_B
                    }
                    # wave B x phase: o (bf16) + f (fp8 DR)
                    for p in range(XP):
                        for kt in (2 * p, 2 * p + 1):
                            w = wxb_ap(2, ut, kt)
                            for ti in range(2):
                                nc.tensor.matmul(
                                    psB[2][ti][:], w, xab[ti](kt),
                                    start=(kt == 0), stop=False,
                                )
                        wq = wxq_ap(0, ut, p)
                        for ti in range(2):
                            nc.tensor.matmul(
                                psB[0][ti][:], wq, x8ab[ti](p),
                                start=(p == 0), stop=False, perf_mode=DR,
                            )
                    # wave A h phase
                    for p in range(HP):
                        for kt in (2 * p, 2 * p + 1):
                            w = whb_ap(ut, kt)
                            for ti in range(2):
                                nc.tensor.matmul(
                                    psA[3][ti][:], w, hab[ti](kt),
                                    start=False, stop=(kt == KH - 1),
                                )
                        wq = whq_ap(1, ut, p)
                        for ti in range(2):
                            nc.tensor.matmul(
                                psA[1][ti][:], wq, h8ab[ti](p),
                                start=False, stop=(p == HP - 1), perf_mode=DR,
                            )
                    ig_a = stage1(psA, 0, ut, f"a{ut}")
                    ig_b = stage1(psA, 1, ut, f"b{ut}")
                    # wave B h phase: o bf16 (6 pairs) + f DR + o8 DR tail
                    for p in range(HP):
                        if p < OBH // 2:
                            for kt in (2 * p, 2 * p + 1):
                                w = who_ap(ut, kt)
                                for ti in range(2):
                                    nc.tensor.matmul(
                                        psB[2][ti][:], w, hab[ti](kt),
                                        start=False, stop=False,
                                    )
                        wq = whq_ap(0, ut, p)
                        for ti in range(2):
                            nc.tensor.matmul(
                                psB[0][ti][:], wq, h8ab[ti](p),
                                start=False, stop=(p == HP - 1), perf_mode=DR,
                            )
                        if p >= OBH // 2:
                            j = p - OBH // 2
                            co = ut * 128
                            wq8 = who8_t[:, 2 * j : 2 * j + 2, co : co + 128]
                            for ti in range(2):
                                nc.tensor.matmul(
                                    psB[2][ti][:], wq8,
                                    h8ab[ti](HP - (KH - OBH) // 2 + j),
                                    start=False,
                                    stop=(j == (KH - OBH) // 2 - 1),
                                    perf_mode=DR,
                                )
                    stage2(psB, 0, ig_a, na, ut)
                    stage2(psB, 1, ig_b, nb, ut)
    _split_excess_waits(nc)
    return nc


_NC_CACHE = None


def _get_nc():
    global _NC_CACHE
    if _NC_CACHE is None:
        _NC_CACHE = build_nc()
    return _NC_CACHE


def make_in_maps(x, h, c, Wxf, Wxi, Wxo, Wxg, bf, bi, bo, bg, Whf, Whi, Who, Whg):
    bf16 = ml_dtypes.bfloat16
    fp8 = ml_dtypes.float8_e4m3
    xT_f = np.ascontiguousarray(np.asarray(x, np.float32).T)
    hT_f = np.ascontiguousarray(np.asarray(h, np.float32).T)
    xT = xT_f.astype(bf16)
    hT = hT_f.astype(bf16)
    xT8 = (xT_f * (1.0 / S8)).astype(fp8)
    hT8 = (hT_f * (1.0 / S8)).astype(fp8)
    c = np.asarray(c, np.float32)
    Wxo_f = np.asarray(Wxo, np.float32)
    Wxg_f = np.asarray(Wxg, np.float32)
    Who_f = np.asarray(Who, np.float32)
    Whg_f = np.asarray(Whg, np.float32)
    Wxf_f = np.asarray(Wxf, np.float32)
    Wxi_f = np.asarray(Wxi, np.float32)
    Whf_f = np.asarray(Whf, np.float32)
    Whi_f = np.asarray(Whi, np.float32)
    bias = np.stack([np.asarray(v, np.float32) for v in (bf, bi, bo, bg)])

    in_maps = []
    for i in range(N_CORES):
        s = slice(i * US, (i + 1) * US)
        wxb_i = np.concatenate([Wxo_f[:, s], Wxg_f[:, s]], axis=1).astype(bf16)
        whb_i = np.concatenate([Who_f[:, s], Whg_f[:, s]], axis=1).astype(bf16)
        wxq_i = (np.concatenate([Wxf_f[:, s], Wxi_f[:, s]], axis=1) * S8).astype(fp8)
        whq_i = (np.concatenate([Whf_f[:, s], Whi_f[:, s]], axis=1) * S8).astype(fp8)
        who8_i = (Who_f[OBH * 128 :, s] * S8).astype(fp8)
        b_i = np.concatenate([bias[g, s] for g in range(4)])  # [1024]
        b_i = np.ascontiguousarray(b_i.reshape(4 * UT, 128).T)  # [128, 8]
        cT_i = np.ascontiguousarray(c[:, s].T).astype(bf16)  # [US, B]
        in_maps.append(
            {
                "xT": xT, "hT": hT, "xT8": xT8, "hT8": hT8,
                "wxb": wxb_i, "whb": whb_i, "wxq": wxq_i, "whq": whq_i,
                "who8": who8_i, "bias": b_i, "cT": cT_i,
            }
        )
    return in_maps


def run(in_maps, **kwargs):
    nc = _get_nc()
    return run_bass_kernel_spmd(nc, in_maps, list(range(N_CORES)), **kwargs)


def gather(results):
    h_new = np.empty((B, U), np.float32)
    c_new = np.empty((B, U), np.float32)
    for i in range(N_CORES):
        s = slice(i * US, (i + 1) * US)
        h_new[:, s] = results[i]["h_newT"].astype(np.float32).T
        c_new[:, s] = results[i]["c_newT"].astype(np.float32).T
    return h_new, c_new


def kernel(**inputs):
    res = run(make_in_maps(**inputs))
    return gather(res.results)


# revision 30
# speedup vs baseline: 1.1889x; 1.1889x over previous
"""LSTM cell (B=4096, D=U=2048) on 8 trn2 NeuronCores.

Tensor-parallel over units: core i computes units [i*256,(i+1)*256) of every
gate. Per core:
    z^T[1024 units, 4096 batch] = Wx_shard^T @ x^T + Wh_shard^T @ h^T
accumulated in PSUM, gate activations fused with the bias add on ScalarE
(units on partitions -> bias is per-partition), elementwise LSTM combine on
VectorE, outputs stored transposed and re-transposed on the host.

v21: per-gate precision allocation. Quantization noise injected into a
gate's pre-activation is amplified ~6.5x more by g (tanh, unit slope) than
by i (sigmoid, slope <= 1/4, times tanh'(c_new) attenuation), so the fp8
budget goes where it is cheap: f and i run their entire 4096-deep
contraction in fp8 e4m3 DoubleRow (2 k-tiles per PE pass), o runs bf16 with
its last 6 h k-tiles fp8, g stays all-bf16. The summed per-gate noise
variances land at rel err 1.934e-2 (budget 2e-2, exact numpy sim of the
quantization matches hardware to 4 digits), while the matmul instruction
count drops from 29 to 23.25 per (gate,ut,btile) on average.

Schedule: the DMA engine delivers only ~115GB/s for its first ~13us (it
has its own ramp), so batch tile 0 opens with the f/i fp8 x-phase, whose
head (x8 + wxq) is written in 256-512KB slice-granular pieces — Tile
tracks AP-overlap deps, so the first DoubleRow pair starts after the first
512KB instead of the whole 2.5MB head — and its ~7us of work gives the
bf16 x+weight stream a head start; the bf16 k-outer then tracks chunk
arrivals across all 8 PSUM banks with HAM staying at full rate end-to-end. Tiles 2..7 run in weight-stationary pairs with two 4-bank PSUM
waves (g,i then f,o) so wave evacuation overlaps the other wave's matmuls;
the final o-gate is split 384/128 so only a 128-wide act+mul+store trails
the last matmul. DMA dispatch costs ~0.7us per descriptor-set on an engine
queue, so loads are batched (4-ktile chunks) and split across the two HWDGE
queues by consumption time: sync carries the fp8 x head, then the n0 act
chunks with the x weights interleaved right behind their chunks (arrival
order == consumption order on one FIFO, robust to cross-queue bandwidth
races), then the late-consumed fp8 h tail (whq/who8/h8); scalar carries the remaining
weight stream (it is idle until the first elementwise); per-batch-tile act
loads all ride sync, since scalar-queue dispatches would serialize behind
stage activations.
"""

import sys

sys.path.insert(0, "/opt/trn_rl_repo")

import ml_dtypes
import numpy as np

import concourse.bass as bass
import concourse.mybir as mybir
import concourse.tile as tile
from concourse.bass_utils import run_bass_kernel_spmd

B, D, U = 4096, 2048, 2048
N_CORES = 8
US = U // N_CORES          # units per core per gate (256)
UT = US // 128             # unit tiles of 128 per gate (2)
NB = 512                   # batch tile (free dim)
NT = B // NB               # batch tiles (8)
KX = D // 128              # k tiles for x gemm (16)
KH = U // 128              # k tiles for h gemm (16)
XP = KX // 2               # fp8 DoubleRow pairs over x (8)
HP = KH // 2               # fp8 DoubleRow pairs over h (8)
OBH = 10                   # o-gate bf16 h k-tiles; tiles 10..15 run fp8
OBW = 12                   # whb batch coverage ([o|g] cols); g-only beyond
BF16 = mybir.dt.bfloat16
FP8 = mybir.dt.float8e4
F32 = mybir.dt.float32
AF = mybir.ActivationFunctionType
DR = mybir.MatmulPerfMode.DoubleRow
S8 = 8.0                   # symmetric fp8 scale: act/8 and W*8 (cancels)

# gate index in the elementwise/bias layout: f=0, i=1, o=2, g=3
GATE_ORDER = (3, 1, 0, 2)  # consumption order g, i, f, o


def _split_excess_waits(nc, maxw=1):
    """This walrus build rejects instructions carrying more than one sem-wait
    ("Too many sync wait commands"), but Tile freely attaches several. Hoist
    the extra waits onto same-engine nops inserted right before the
    instruction — engine streams are in-order, so blocking semantics are
    identical."""
    cnt = 0
    for fn in nc.m.functions:
        for bb in fn.blocks:
            new_insts = []
            for inst in bb.instructions:
                si = inst.sync_info
                waits = list(si.on_wait) if si is not None else []
                if len(waits) > maxw:
                    for i in range(0, len(waits) - maxw, maxw):
                        nop = mybir.InstNoOp(name=f"syncsplit-{cnt}")
                        cnt += 1
                        nop.engine = inst.engine
                        nop.sync_info = mybir.SyncInfo(
                            on_wait=waits[i : i + maxw], on_update=[]
                        )
                        new_insts.append(nop)
                    si.on_wait = waits[len(waits) - maxw :]
                new_insts.append(inst)
            if len(new_insts) != len(bb.instructions):
                bb.instructions = new_insts
    return cnt


def build_nc() -> bass.Bass:
    nc = bass.Bass()

    xT = nc.dram_tensor("xT", [D, B], BF16, kind="ExternalInput")
    hT = nc.dram_tensor("hT", [U, B], BF16, kind="ExternalInput")
    xT8 = nc.dram_tensor("xT8", [D, B], FP8, kind="ExternalInput")
    hT8 = nc.dram_tensor("hT8", [U, B], FP8, kind="ExternalInput")
    # bf16 weights, columns [o | g] (256 each); o rows used only for kt<OBH
    wxb = nc.dram_tensor("wxb", [D, 2 * US], BF16, kind="ExternalInput")
    whb = nc.dram_tensor("whb", [U, 2 * US], BF16, kind="ExternalInput")
    # fp8 weights, columns [f | i]; host pre-scales W*8, act/8 (exact cancel)
    wxq = nc.dram_tensor("wxq", [D, 2 * US], FP8, kind="ExternalInput")
    whq = nc.dram_tensor("whq", [U, 2 * US], FP8, kind="ExternalInput")
    who8 = nc.dram_tensor("who8", [(KH - OBH) * 128, US], FP8, kind="ExternalInput")
    # bias, host-prepped to [128, 8]: column j = units [j*128,(j+1)*128) of
    # the concatenated [f,i,o,g] 1024-unit block (gate j//2, unit-tile j%2)
    bias = nc.dram_tensor("bias", [128, 4 * UT], F32, kind="ExternalInput")
    # c input and both outputs travel as bf16: halves DMA traffic and the
    # final output drain; costs ~1e-3 rel err (budget is 2e-2)
    cT = nc.dram_tensor("cT", [US, B], BF16, kind="ExternalInput")
    h_newT = nc.dram_tensor("h_newT", [US, B], BF16, kind="ExternalOutput")
    c_newT = nc.dram_tensor("c_newT", [US, B], BF16, kind="ExternalOutput")

    xT_r = xT.rearrange("(kt p) b -> p kt b", p=128)    # [128, KX, B]
    hT_r = hT.rearrange("(kt p) b -> p kt b", p=128)
    xT8_r = xT8.rearrange("(kt p) b -> p kt b", p=128)
    hT8_r = hT8.rearrange("(kt p) b -> p kt b", p=128)
    wxb_r = wxb.rearrange("(kt p) u -> p kt u", p=128)  # [128, KX, 512]
    whb_r = whb.rearrange("(kt p) u -> p kt u", p=128)
    wxq_r = wxq.rearrange("(kt p) u -> p kt u", p=128)
    whq_r = whq.rearrange("(kt p) u -> p kt u", p=128)
    who8_r = who8.rearrange("(kt p) u -> p kt u", p=128)  # [128, 4, US]

    XCH = [(0, 1), (1, 2), (2, 4), (4, 8), (8, 12), (12, 16)]
    HCH = [(0, 4), (4, 8), (8, 12), (12, 16)]

    with tile.TileContext(nc) as tc:
        with (
            tc.tile_pool(name="wpool", bufs=1) as wpool,
            tc.tile_pool(name="singles", bufs=1) as singles,
            tc.tile_pool(name="acts", bufs=2) as apool,
            tc.tile_pool(name="ew", bufs=2) as epool,
            tc.tile_pool(name="psum", bufs=8, space="PSUM") as ppool,
        ):
            # --- weight column helpers. Weights stream in 4-ktile batches
            # on the Activation-engine HWDGE queue so their dispatch cost
            # never serializes with the activation loads on the sync queue.
            wxb0_t = [None] * 4           # kt 0..3 as single-kt tiles
            wxbB = [None] * (KX // 4)
            whbB = [None] * (OBW // 4)
            whgB = [None]                 # g-only cols for kt >= OBH
            wxqB = [None] * 2
            whqB = [None] * 2

            def bcol(gi, ut):  # o,g in wxb/whb
                return (0 if gi == 2 else US) + ut * 128

            def qcol(gi, ut):  # f,i in wxq/whq
                return (0 if gi == 0 else US) + ut * 128

            def wxb_ap(gi, ut, kt):
                c0 = bcol(gi, ut)
                if kt < 4:
                    return wxb0_t[kt][:, c0 : c0 + 128]
                return wxbB[kt // 4][:, kt % 4, c0 : c0 + 128]

            def whb_ap(ut, kt):  # g-gate h weight
                if kt < OBW:
                    c0 = bcol(3, ut)
                    return whbB[kt // 4][:, kt % 4, c0 : c0 + 128]
                c0 = ut * 128
                return whgB[0][:, kt - OBW, c0 : c0 + 128]

            def who_ap(ut, kt):  # o-gate bf16 h weight, kt < OBH
                c0 = bcol(2, ut)
                return whbB[kt // 4][:, kt % 4, c0 : c0 + 128]

            def wxq_ap(gi, ut, p):
                c0 = qcol(gi, ut)
                return wxqB[p // 4][:, 2 * (p % 4) : 2 * (p % 4) + 2, c0 : c0 + 128]

            def whq_ap(gi, ut, p):
                c0 = qcol(gi, ut)
                return whqB[p // 4][:, 2 * (p % 4) : 2 * (p % 4) + 2, c0 : c0 + 128]

            # --- startup DMA stream (single HWDGE ring, FIFO) in PE
            # consumption order: bf16 x chunks + their weights, then the fp8
            # x copy + weights, then h likewise.
            # the DMA engine runs at ~1/3 rate for its first ~13us, so the
            # fp8 head is written in slice-granular pieces: the first DR
            # pair needs only the first 512KB, not the whole 2.5MB head
            x8_0 = apool.tile([128, KX, NB], FP8, tag="x8_sb", name="x8_0")
            wxqB[0] = wpool.tile([128, 8, 2 * US], FP8, tag="wxqB0", name="wxqB0")
            wxqB[1] = wpool.tile([128, 8, 2 * US], FP8, tag="wxqB1", name="wxqB1")
            nc.sync.dma_start(out=x8_0[:, 0:2, :], in_=xT8_r[:, 0:2, bass.ts(0, NB)])
            nc.sync.dma_start(out=wxqB[0][:, 0:2, :], in_=wxq_r[:, 0:2, :])
            nc.sync.dma_start(out=x8_0[:, 2:4, :], in_=xT8_r[:, 2:4, bass.ts(0, NB)])
            nc.sync.dma_start(out=wxqB[0][:, 2:4, :], in_=wxq_r[:, 2:4, :])
            nc.sync.dma_start(out=x8_0[:, 4:8, :], in_=xT8_r[:, 4:8, bass.ts(0, NB)])
            nc.sync.dma_start(out=wxqB[0][:, 4:8, :], in_=wxq_r[:, 4:8, :])
            nc.sync.dma_start(out=x8_0[:, 8:12, :], in_=xT8_r[:, 8:12, bass.ts(0, NB)])
            nc.sync.dma_start(out=wxqB[1][:, 0:4, :], in_=wxq_r[:, 8:12, :])
            nc.sync.dma_start(out=x8_0[:, 12:16, :], in_=xT8_r[:, 12:16, bass.ts(0, NB)])
            nc.sync.dma_start(out=wxqB[1][:, 4:8, :], in_=wxq_r[:, 12:16, :])
            x0c = {}
            for (k0, k1) in XCH:
                xc = apool.tile(
                    [128, k1 - k0, NB], BF16, tag=f"xbc{k0}", bufs=1, name=f"x0c{k0}"
                )
                nc.sync.dma_start(out=xc[:], in_=xT_r[:, k0:k1, bass.ts(0, NB)])
                for kt in range(k0, k1):
                    x0c[kt] = xc[:, kt - k0, :]
                # x weights ride the same FIFO right behind their chunk, so
                # arrival order equals consumption order with no cross-queue
                # bandwidth race
                if k1 <= 2:
                    for kt in range(k0, k1):
                        wt = wpool.tile([128, 2 * US], BF16, tag=f"wxb0{kt}")
                        nc.sync.dma_start(out=wt[:], in_=wxb_r[:, kt, :])
                        wxb0_t[kt] = wt
                elif k1 <= 4:
                    wt = wpool.tile([128, 2, 2 * US], BF16, tag="wxb023")
                    nc.sync.dma_start(out=wt[:], in_=wxb_r[:, 2:4, :])
                    wxb0_t[2] = wt[:, 0, :]
                    wxb0_t[3] = wt[:, 1, :]
                else:
                    j = k0 // 4
                    wt = wpool.tile(
                        [128, 4, 2 * US], BF16, tag=f"wxbB{j}", name=f"wxbB{j}"
                    )
                    nc.sync.dma_start(
                        out=wt[:, 0:2, :], in_=wxb_r[:, k0 : k0 + 2, :]
                    )
                    nc.sync.dma_start(
                        out=wt[:, 2:4, :], in_=wxb_r[:, k0 + 2 : k0 + 4, :]
                    )
                    wxbB[j] = wt
            h0c = {}
            for (k0, k1) in HCH:
                hc = apool.tile(
                    [128, k1 - k0, NB], BF16, tag=f"hbc{k0}", bufs=1, name=f"h0c{k0}"
                )
                nc.sync.dma_start(out=hc[:], in_=hT_r[:, k0:k1, bass.ts(0, NB)])
                for kt in range(k0, k1):
                    h0c[kt] = hc[:, kt - k0, :]
            # remaining (later-consumed) weights stream on the scalar queue
            b_sb = singles.tile([128, 4 * UT], F32)
            nc.scalar.dma_start(out=b_sb[:], in_=bias[:])
            for j in range(OBW // 4):
                wt = wpool.tile([128, 4, 2 * US], BF16, tag=f"whbB{j}")
                nc.scalar.dma_start(out=wt[:], in_=whb_r[:, 4 * j : 4 * j + 4, :])
                whbB[j] = wt
            wt = wpool.tile([128, KH - OBW, US], BF16, tag="whgB")
            nc.scalar.dma_start(out=wt[:], in_=whb_r[:, OBW:KH, US : 2 * US])
            whgB[0] = wt
            # the fp8 h-phase tail (i/f h weights, o8 weights, h8 acts) is
            # consumed last in n0; it rides the sync queue, which drains its
            # startup work ~20us before the PE needs these
            for j in range(2):
                wt = wpool.tile([128, 8, 2 * US], FP8, tag=f"whqB{j}")
                nc.sync.dma_start(out=wt[:], in_=whq_r[:, 8 * j : 8 * j + 8, :])
                whqB[j] = wt
            who8_t = wpool.tile([128, KH - OBH, US], FP8, tag="who8")
            nc.sync.dma_start(out=who8_t[:], in_=who8_r[:])
            h8_0 = apool.tile([128, KH, NB], FP8, tag="h8_sb", name="h8_0")
            nc.sync.dma_start(out=h8_0[:], in_=hT8_r[:, :, bass.ts(0, NB)])

            # --- matmul emitters. Moving operands come via accessors:
            # xb(kt)/hb(kt) -> [128, NB] bf16, x8(p)/h8(p) -> [128, 2, NB].
            def mm_g(ps, ut, xb, hb, cols=slice(0, NB)):
                for kt in range(KX):
                    nc.tensor.matmul(
                        ps, wxb_ap(3, ut, kt), xb(kt)[:, cols],
                        start=(kt == 0), stop=False,
                    )
                for kt in range(KH):
                    nc.tensor.matmul(
                        ps, whb_ap(ut, kt), hb(kt)[:, cols],
                        start=False, stop=(kt == KH - 1),
                    )

            def mm_o_x(ps, ut, xb, cols=slice(0, NB)):
                for kt in range(KX):
                    nc.tensor.matmul(
                        ps, wxb_ap(2, ut, kt), xb(kt)[:, cols],
                        start=(kt == 0), stop=False,
                    )

            def mm_o_hb(ps, ut, hb, cols=slice(0, NB)):
                for kt in range(OBH):
                    nc.tensor.matmul(
                        ps, who_ap(ut, kt), hb(kt)[:, cols],
                        start=False, stop=False,
                    )

            def mm_o_h(ps, ut, hb, h8, cols=slice(0, NB)):
                mm_o_hb(ps, ut, hb, cols)
                mm_o8(ps, ut, h8, cols)

            def mm_o8(ps, ut, h8, cols=slice(0, NB)):
                co = ut * 128
                for j in range((KH - OBH) // 2):
                    nc.tensor.matmul(
                        ps,
                        who8_t[:, 2 * j : 2 * j + 2, co : co + 128],
                        h8(HP - (KH - OBH) // 2 + j)[:, :, cols],
                        start=False, stop=(j == (KH - OBH) // 2 - 1),
                        perf_mode=DR,
                    )

            def mm_q_x(ps, gi, ut, x8, cols=slice(0, NB)):
                for p in range(XP):
                    nc.tensor.matmul(
                        ps, wxq_ap(gi, ut, p), x8(p)[:, :, cols],
                        start=(p == 0), stop=False, perf_mode=DR,
                    )

            def mm_q_h(ps, gi, ut, h8, cols=slice(0, NB)):
                for p in range(HP):
                    nc.tensor.matmul(
                        ps, whq_ap(gi, ut, p), h8(p)[:, :, cols],
                        start=False, stop=(p == HP - 1), perf_mode=DR,
                    )

            def act_gate(ps, gi, ut, name, w=NB):
                # o-gate tiles are bf16 (they feed the bf16 h_new output);
                # f/i/g stay fp32 for the c_new accumulate path
                dt = BF16 if gi == 2 else F32
                g_sb = epool.tile([128, w], dt, tag=f"gate{gi}_{w}", name=name)
                nc.scalar.activation(
                    g_sb[:],
                    ps[:],
                    AF.Tanh if gi == 3 else AF.Sigmoid,
                    bias=b_sb[:, gi * UT + ut : gi * UT + ut + 1],
                )
                return g_sb

            def elementwise(pss, n, ut):
                # pss indexed by gi; groups complete in GATE_ORDER, so
                # evaluate the LSTM chain in that order
                nsl = bass.ts(n, NB)
                usl = slice(ut * 128, (ut + 1) * 128)
                c_sb = epool.tile([128, NB], BF16, tag="c_sb", name="c_sb")
                nc.sync.dma_start(out=c_sb[:], in_=cT[usl, nsl])
                g_t = act_gate(pss[3], 3, ut, "g_t")
                i_t = act_gate(pss[1], 1, ut, "i_t")
                nc.vector.tensor_mul(i_t[:], i_t[:], g_t[:])      # i*g
                f_t = act_gate(pss[0], 0, ut, "f_t")
                nc.vector.tensor_mul(f_t[:], f_t[:], c_sb[:])     # f*c
                cn = epool.tile([128, NB], BF16, tag="cn", name="cn")
                nc.vector.tensor_add(cn[:], f_t[:], i_t[:])       # c_new
                nc.sync.dma_start(out=c_newT[usl, nsl], in_=cn[:])
                tn = epool.tile([128, NB], BF16, tag="tnb", name="tn")
                nc.scalar.activation(tn[:], cn[:], AF.Tanh)       # tanh(c_new)
                o_t = act_gate(pss[2], 2, ut, "o_t")
                nc.vector.tensor_mul(o_t[:], o_t[:], tn[:])       # h_new
                nc.sync.dma_start(out=h_newT[usl, nsl], in_=o_t[:])

            def stage1(psA, ti, ut, tname):
                # after wave A (g,i) stops: compute ig = sigmoid(i)*tanh(g),
                # freeing wave A's PSUM banks while wave B still matmuls
                g_t = epool.tile([128, NB], F32, tag="gate3", name=f"g_{tname}")
                nc.scalar.activation(
                    g_t[:], psA[3][ti][:], AF.Tanh,
                    bias=b_sb[:, 3 * UT + ut : 3 * UT + ut + 1],
                )
                ig = epool.tile([128, NB], F32, tag="ig", name=f"ig_{tname}")
                nc.scalar.activation(
                    ig[:], psA[1][ti][:], AF.Sigmoid,
                    bias=b_sb[:, 1 * UT + ut : 1 * UT + ut + 1],
                )
                nc.vector.tensor_mul(ig[:], ig[:], g_t[:])
                return ig

            def stage2(psB, ti, ig, n, ut):
                # after wave B (f,o) stops: finish the LSTM combine
                nsl = bass.ts(n, NB)
                usl = slice(ut * 128, (ut + 1) * 128)
                c_sb = epool.tile([128, NB], BF16, tag="c_sb", name="c_sb")
                nc.sync.dma_start(out=c_sb[:], in_=cT[usl, nsl])
                f_t = act_gate(psB[0][ti], 0, ut, "f_t")
                nc.vector.tensor_mul(f_t[:], f_t[:], c_sb[:])     # f*c
                cn = epool.tile([128, NB], BF16, tag="cn", name="cn")
                nc.vector.tensor_add(cn[:], f_t[:], ig[:])        # c_new
                nc.sync.dma_start(out=c_newT[usl, nsl], in_=cn[:])
                tn = epool.tile([128, NB], BF16, tag="tnb", name="tn")
                nc.scalar.activation(tn[:], cn[:], AF.Tanh)       # tanh(c_new)
                o_t = act_gate(psB[2][ti], 2, ut, "o_t")
                nc.vector.tensor_mul(o_t[:], o_t[:], tn[:])       # h_new
                nc.sync.dma_start(out=h_newT[usl, nsl], in_=o_t[:])

            # --- n = 0: k-outer over the bf16 gates across all 8 PSUM banks
            # so the PE tracks the chunked arrival stream; the fp8 gates'
            # DoubleRow matmuls run after (their whole-tile deps have landed
            # by then).
            ps_all = [
                [
                    ppool.tile([128, NB], F32, tag="ps", name=f"ps{ut}{gi}")
                    for gi in range(4)
                ]
                for ut in range(UT)
            ]
            xb0 = lambda kt: x0c[kt]
            hb0 = lambda kt: h0c[kt]
            x80 = lambda p: x8_0[:, 2 * p : 2 * p + 2, :]
            h80 = lambda p: h8_0[:, 2 * p : 2 * p + 2, :]
            # fp8 x-phase first: it needs only 1.5MB to start, and its
            # ~7us of work gives the bf16 x+weight FIFO a head start so the
            # k-outer phase below never catches the stream
            for p in range(XP):
                for gi in (1, 0):
                    for ut in range(UT):
                        nc.tensor.matmul(
                            ps_all[ut][gi][:],
                            wxq_ap(gi, ut, p), x80(p),
                            start=(p == 0), stop=False, perf_mode=DR,
                        )
            for kt in range(KX):
                for ut in range(UT):
                    nc.tensor.matmul(
                        ps_all[ut][3][:], wxb_ap(3, ut, kt), x0c[kt],
                        start=(kt == 0), stop=False,
                    )
                for ut in range(UT):
                    nc.tensor.matmul(
                        ps_all[ut][2][:], wxb_ap(2, ut, kt), x0c[kt],
                        start=(kt == 0), stop=False,
                    )
            for kt in range(OBH):
                for ut in range(UT):
                    nc.tensor.matmul(
                        ps_all[ut][3][:], whb_ap(ut, kt), h0c[kt],
                        start=False, stop=False,
                    )
                for ut in range(UT):
                    nc.tensor.matmul(
                        ps_all[ut][2][:], who_ap(ut, kt), h0c[kt],
                        start=False, stop=False,
                    )
            for kt in range(OBH, KH):
                for ut in range(UT):
                    nc.tensor.matmul(
                        ps_all[ut][3][:], whb_ap(ut, kt), h0c[kt],
                        start=False, stop=(kt == KH - 1),
                    )
            for gi in (1, 0):
                for ut in range(UT):
                    mm_q_h(ps_all[ut][gi][:], gi, ut, h80)
            for ut in range(UT):
                mm_o8(ps_all[ut][2][:], ut, h80)

            # --- n = 1 loads issued before n0's elementwise DMAs so they
            # don't queue behind the output drains on the ring. bf16 x/h
            # reuse the n0 chunk buffers (their readers are all emitted);
            # fp8 copies take the second buffer of the pair tags.
            x1c = {}
            for (k0, k1) in XCH:
                xc = apool.tile(
                    [128, k1 - k0, NB], BF16, tag=f"xbc{k0}", bufs=1, name=f"x1c{k0}"
                )
                nc.sync.dma_start(out=xc[:], in_=xT_r[:, k0:k1, bass.ts(1, NB)])
                for kt in range(k0, k1):
                    x1c[kt] = xc[:, kt - k0, :]
            h1c = {}
            for (k0, k1) in HCH:
                hc = apool.tile(
                    [128, k1 - k0, NB], BF16, tag=f"hbc{k0}", bufs=1, name=f"h1c{k0}"
                )
                nc.sync.dma_start(out=hc[:], in_=hT_r[:, k0:k1, bass.ts(1, NB)])
                for kt in range(k0, k1):
                    h1c[kt] = hc[:, kt - k0, :]
            x8_1 = apool.tile([128, KX, NB], FP8, tag="x8_sb", name="x8_1")
            nc.sync.dma_start(out=x8_1[:], in_=xT8_r[:, :, bass.ts(1, NB)])
            h8_1 = apool.tile([128, KH, NB], FP8, tag="h8_sb", name="h8_1")
            nc.sync.dma_start(out=h8_1[:], in_=hT8_r[:, :, bass.ts(1, NB)])

            for ut in range(UT):
                elementwise(ps_all[ut], 0, ut)

            xb1 = lambda kt: x1c[kt]
            hb1 = lambda kt: h1c[kt]
            x81 = lambda p: x8_1[:, 2 * p : 2 * p + 2, :]
            h81 = lambda p: h8_1[:, 2 * p : 2 * p + 2, :]
            for ut in range(UT):
                pss = [
                    ppool.tile([128, NB], F32, tag="ps", name=f"ps{gi}")
                    for gi in range(4)
                ]
                mm_g(pss[3][:], ut, xb1, hb1)
                mm_q_x(pss[1][:], 1, ut, x81)
                mm_q_h(pss[1][:], 1, ut, h81)
                mm_q_x(pss[0][:], 0, ut, x81)
                mm_q_h(pss[0][:], 0, ut, h81)
                mm_o_x(pss[2][:], ut, xb1)
                mm_o_h(pss[2][:], ut, hb1, h81)
                elementwise(pss, 1, ut)

            # --- n = 2..7 in weight-stationary pairs: per stationary weight
            # slice, both batch tiles' matmuls run back-to-back (walrus skips
            # the second LDWEIGHTS). Two 4-bank PSUM waves per ut: A=(g,i),
            # B=(f,o); both waves' x phases run first so the pair's x tiles
            # die early enough for the next pair's prefetch.
            WAVE_A = (3, 1)
            WAVE_B = (0, 2)
            for (na, nb) in ((2, 3), (4, 5), (6, 7)):
                last_pair = nb == NT - 1
                xa_t = apool.tile([128, KX, NB], BF16, tag="x_sb", name=f"x{na}")
                nc.sync.dma_start(out=xa_t[:], in_=xT_r[:, :, bass.ts(na, NB)])
                x8a_t = apool.tile([128, KX, NB], FP8, tag="x8_sb", name=f"x8{na}")
                nc.sync.dma_start(out=x8a_t[:], in_=xT8_r[:, :, bass.ts(na, NB)])
                xb_t = apool.tile([128, KX, NB], BF16, tag="x_sb", name=f"x{nb}")
                nc.sync.dma_start(out=xb_t[:], in_=xT_r[:, :, bass.ts(nb, NB)])
                x8b_t = apool.tile([128, KX, NB], FP8, tag="x8_sb", name=f"x8{nb}")
                nc.sync.dma_start(out=x8b_t[:], in_=xT8_r[:, :, bass.ts(nb, NB)])
                ha_t = apool.tile([128, KH, NB], BF16, tag="h_sb", name=f"h{na}")
                nc.sync.dma_start(out=ha_t[:], in_=hT_r[:, :, bass.ts(na, NB)])
                h8a_t = apool.tile([128, KH, NB], FP8, tag="h8_sb", name=f"h8{na}")
                nc.sync.dma_start(out=h8a_t[:], in_=hT8_r[:, :, bass.ts(na, NB)])
                hb_t = apool.tile([128, KH, NB], BF16, tag="h_sb", name=f"h{nb}")
                nc.sync.dma_start(out=hb_t[:], in_=hT_r[:, :, bass.ts(nb, NB)])
                h8b_t = apool.tile([128, KH, NB], FP8, tag="h8_sb", name=f"h8{nb}")
                nc.sync.dma_start(out=h8b_t[:], in_=hT8_r[:, :, bass.ts(nb, NB)])
                xab = [lambda kt, t=xa_t: t[:, kt, :], lambda kt, t=xb_t: t[:, kt, :]]
                hab = [lambda kt, t=ha_t: t[:, kt, :], lambda kt, t=hb_t: t[:, kt, :]]
                x8ab = [
                    lambda p, t=x8a_t: t[:, 2 * p : 2 * p + 2, :],
                    lambda p, t=x8b_t: t[:, 2 * p : 2 * p + 2, :],
                ]
                h8ab = [
                    lambda p, t=h8a_t: t[:, 2 * p : 2 * p + 2, :],
                    lambda p, t=h8b_t: t[:, 2 * p : 2 * p + 2, :],
                ]

                for ut in range(UT):
                    if last_pair and ut == UT - 1:
                        # kernel finale: per-tile gate-outer; tile b's o-gate
                        # is split 384/128 so only a 128-wide act+mul+store
                        # trails the last matmul
                        usl = slice(ut * 128, (ut + 1) * 128)
                        pss = [
                            ppool.tile([128, NB], F32, tag="ps", name=f"ps{gi}")
                            for gi in range(4)
                        ]
                        mm_g(pss[3][:], ut, xab[0], hab[0])
                        mm_q_x(pss[1][:], 1, ut, x8ab[0])
                        mm_q_h(pss[1][:], 1, ut, h8ab[0])
                        mm_q_x(pss[0][:], 0, ut, x8ab[0])
                        mm_q_h(pss[0][:], 0, ut, h8ab[0])
                        mm_o_x(pss[2][:], ut, xab[0])
                        mm_o_h(pss[2][:], ut, hab[0], h8ab[0])
                        elementwise(pss, na, ut)

                        nslb = bass.ts(nb, NB)
                        psb = [
                            ppool.tile([128, NB], F32, tag="ps", name=f"psb{gi}")
                            for gi in range(4)
                        ]
                        mm_g(psb[3][:], ut, xab[1], hab[1])
                        mm_q_x(psb[1][:], 1, ut, x8ab[1])
                        mm_q_h(psb[1][:], 1, ut, h8ab[1])
                        mm_q_x(psb[0][:], 0, ut, x8ab[1])
                        mm_q_h(psb[0][:], 0, ut, h8ab[1])
                        # combine chain for c_new runs during the o loops
                        c_sb = epool.tile([128, NB], BF16, tag="c_sb", name="c_sb")
                        nc.sync.dma_start(out=c_sb[:], in_=cT[usl, nslb])
                        g_t = act_gate(psb[3], 3, ut, "g_t")
                        i_t = act_gate(psb[1], 1, ut, "i_t")
                        nc.vector.tensor_mul(i_t[:], i_t[:], g_t[:])
                        f_t = act_gate(psb[0], 0, ut, "f_t")
                        nc.vector.tensor_mul(f_t[:], f_t[:], c_sb[:])
                        cn = epool.tile([128, NB], BF16, tag="cn", name="cn")
                        nc.vector.tensor_add(cn[:], f_t[:], i_t[:])
                        nc.sync.dma_start(out=c_newT[usl, nslb], in_=cn[:])
                        tn = epool.tile([128, NB], BF16, tag="tnb", name="tn")
                        nc.scalar.activation(tn[:], cn[:], AF.Tanh)
                        # o gate, wide part: evacuates while the narrow part
                        # is still matmuling
                        c1 = slice(0, 384)
                        mm_o_x(psb[2][:, c1], ut, xab[1], cols=c1)
                        mm_o_h(psb[2][:, c1], ut, hab[1], h8ab[1], cols=c1)
                        o1 = act_gate(psb[2][:, c1], 2, ut, "o1", w=384)
                        nc.vector.tensor_mul(o1[:], o1[:], tn[:, c1])
                        nc.sync.dma_start(
                            out=h_newT[usl, nb * NB : nb * NB + 384], in_=o1[:]
                        )
                        ps_o2 = ppool.tile([128, NB], F32, tag="ps", name="ps_o2")
                        c2 = slice(384, 512)
                        mm_o_x(ps_o2[:, 0:128], ut, xab[1], cols=c2)
                        mm_o_h(ps_o2[:, 0:128], ut, hab[1], h8ab[1], cols=c2)
                        o2 = act_gate(ps_o2[:, 0:128], 2, ut, "o2", w=128)
                        nc.vector.tensor_mul(o2[:], o2[:], tn[:, c2])
                        nc.sync.dma_start(
                            out=h_newT[usl, nb * NB + 384 : (nb + 1) * NB],
                            in_=o2[:],
                        )
                        continue
                    psA = {
                        gi: [
                            ppool.tile([128, NB], F32, tag="ps", name=f"ps{gi}{t}")
                            for t in "ab"
                        ]
                        for gi in WAVE_A
                    }
                    # wave A x phase: g (bf16) + i (fp8 DR), weight-stationary
                    for p in range(XP):
                        for kt in (2 * p, 2 * p + 1):
                            w = wxb_ap(3, ut, kt)
                            for ti in range(2):
                                nc.tensor.matmul(
                                    psA[3][ti][:], w, xab[ti](kt),
                                    start=(kt == 0), stop=False,
                                )
                        wq = wxq_ap(1, ut, p)
                        for ti in range(2):
                            nc.tensor.matmul(
                                psA[1][ti][:], wq, x8ab[ti](p),
                                start=(p == 0), stop=False, perf_mode=DR,
                            )
                    psB = {
                        gi: [
                            ppool.tile([128, NB], F32, tag="ps", name=f"ps{gi}{t}")
                            for t in "ab"
                        ]
                        for gi in WAVE_B
                    }
                    # wave B x phase: o (bf16) + f (fp8 DR)
                    for p in range(XP):
                        for kt in (2 * p, 2 * p + 1):
                            w = wxb_ap(2, ut, kt)
                            for ti in range(2):
                                nc.tensor.matmul(
                                    psB[2][ti][:], w, xab[ti](kt),
                                    start=(kt == 0), stop=False,
                                )
                        wq = wxq_ap(0, ut, p)
                        for ti in range(2):
                            nc.tensor.matmul(
                                psB[0][ti][:], wq, x8ab[ti](p),
                                start=(p == 0), stop=False, perf_mode=DR,
                            )
                    # wave A h phase
                    for p in range(HP):
                        for kt in (2 * p, 2 * p + 1):
                            w = whb_ap(ut, kt)
                            for ti in range(2):
                                nc.tensor.matmul(
                                    psA[3][ti][:], w, hab[ti](kt),
                                    start=False, stop=(kt == KH - 1),
                                )
                        wq = whq_ap(1, ut, p)
                        for ti in range(2):
                            nc.tensor.matmul(
                                psA[1][ti][:], wq, h8ab[ti](p),
                                start=False, stop=(p == HP - 1), perf_mode=DR,
                            )
                    ig_a = stage1(psA, 0, ut, f"a{ut}")
                    ig_b = stage1(psA, 1, ut, f"b{ut}")
                    # wave B h phase: o bf16 (6 pairs) + f DR + o8 DR tail
                    for p in range(HP):
                        if p < OBH // 2:
                            for kt in (2 * p, 2 * p + 1):
                                w = who_ap(ut, kt)
                                for ti in range(2):
                                    nc.tensor.matmul(
                                        psB[2][ti][:], w, hab[ti](kt),
                                        start=False, stop=False,
                                    )
                        wq = whq_ap(0, ut, p)
                        for ti in range(2):
                            nc.tensor.matmul(
                                psB[0][ti][:], wq, h8ab[ti](p),
                                start=False, stop=(p == HP - 1), perf_mode=DR,
                            )
                        if p >= OBH // 2:
                            j = p - OBH // 2
                            co = ut * 128
                            wq8 = who8_t[:, 2 * j : 2 * j + 2, co : co + 128]
                            for ti in range(2):
                                nc.tensor.matmul(
                                    psB[2][ti][:], wq8,
                                    h8ab[ti](HP - (KH - OBH) // 2 + j),
                                    start=False,
                                    stop=(j == (KH - OBH) // 2 - 1),
                                    perf_mode=DR,
                                )
                    stage2(psB, 0, ig_a, na, ut)
                    stage2(psB, 1, ig_b, nb, ut)
    _split_excess_waits(nc)
    return nc


_NC_CACHE = None


def _get_nc():
    global _NC_CACHE
    if _NC_CACHE is None:
        _NC_CACHE = build_nc()
    return _NC_CACHE


def make_in_maps(x, h, c, Wxf, Wxi, Wxo, Wxg, bf, bi, bo, bg, Whf, Whi, Who, Whg):
    bf16 = ml_dtypes.bfloat16
    fp8 = ml_dtypes.float8_e4m3
    xT_f = np.ascontiguousarray(np.asarray(x, np.float32).T)
    hT_f = np.ascontiguousarray(np.asarray(h, np.float32).T)
    xT = xT_f.astype(bf16)
    hT = hT_f.astype(bf16)
    xT8 = (xT_f * (1.0 / S8)).astype(fp8)
    hT8 = (hT_f * (1.0 / S8)).astype(fp8)
    c = np.asarray(c, np.float32)
    Wxo_f = np.asarray(Wxo, np.float32)
    Wxg_f = np.asarray(Wxg, np.float32)
    Who_f = np.asarray(Who, np.float32)
    Whg_f = np.asarray(Whg, np.float32)
    Wxf_f = np.asarray(Wxf, np.float32)
    Wxi_f = np.asarray(Wxi, np.float32)
    Whf_f = np.asarray(Whf, np.float32)
    Whi_f = np.asarray(Whi, np.float32)
    bias = np.stack([np.asarray(v, np.float32) for v in (bf, bi, bo, bg)])

    in_maps = []
    for i in range(N_CORES):
        s = slice(i * US, (i + 1) * US)
        wxb_i = np.concatenate([Wxo_f[:, s], Wxg_f[:, s]], axis=1).astype(bf16)
        whb_i = np.concatenate([Who_f[:, s], Whg_f[:, s]], axis=1).astype(bf16)
        wxq_i = (np.concatenate([Wxf_f[:, s], Wxi_f[:, s]], axis=1) * S8).astype(fp8)
        whq_i = (np.concatenate([Whf_f[:, s], Whi_f[:, s]], axis=1) * S8).astype(fp8)
        who8_i = (Who_f[OBH * 128 :, s] * S8).astype(fp8)
        b_i = np.concatenate([bias[g, s] for g in range(4)])  # [1024]
        b_i = np.ascontiguousarray(b_i.reshape(4 * UT, 128).T)  # [128, 8]
        cT_i = np.ascontiguousarray(c[:, s].T).astype(bf16)  # [US, B]
        in_maps.append(
            {
                "xT": xT, "hT": hT, "xT8": xT8, "hT8": hT8,
                "wxb": wxb_i, "whb": whb_i, "wxq": wxq_i, "whq": whq_i,
                "who8": who8_i, "bias": b_i, "cT": cT_i,
            }
        )
    return in_maps


def run(in_maps, **kwargs):
    nc = _get_nc()
    return run_bass_kernel_spmd(nc, in_maps, list(range(N_CORES)), **kwargs)


def gather(results):
    h_new = np.empty((B, U), np.float32)
    c_new = np.empty((B, U), np.float32)
    for i in range(N_CORES):
        s = slice(i * US, (i + 1) * US)
        h_new[:, s] = results[i]["h_newT"].astype(np.float32).T
        c_new[:, s] = results[i]["c_newT"].astype(np.float32).T
    return h_new, c_new


def kernel(**inputs):
    res = run(make_in_maps(**inputs))
    return gather(res.results)
